# revision 20
# baseline (speedup 1.0000x reference)
"""Trainium2 Bass kernel for nn_Attention_loss_919123001759.

Contrastive-style loss:
    H = concat(f1, f2)               [N=8192, D=1024], rows L2-normalized
    e = exp(H @ H.T / t)             [N, N], t = 0.05
    num_i = sum_j e_ij * (S2_ij + eps) * cat_ij
    den_i = sum_j e_ij * negmask_ij          (negmask excludes j=i, |i-j|=B)
    loss = -mean(log(num_i / den_i))

Distribution: rows of H sharded across 8 NeuronCores (1024 rows each). Each
core computes its [1024, 8192] similarity block against the full H^T, fused
exp + masked row-reductions, and outputs per-row partial numerator /
denominator sums. The host adds the analytically-known "special" terms
(diagonal j=i and positive pair j=i+-B, computed in float64 from the raw
features) and performs the final log/mean.

Key tricks:
  - fp8(e4m3) DoubleRow matmul (0.5 cycles/row, ~150 TF/s measured). Only
    the special columns need more precision than fp8 gives, and those are
    excluded on-chip and added host-side in float64. Bulk |sim| <~ 0.2, and
    quantization bias cancels between numerator and denominator.
  - per-core column rotation (host-side gather) so the special diagonal
    lands at compile-time-known tiles -> one SPMD program for all 8 cores.
  - w = (S+eps)*cat precomputed on host (bf16) with zeroed special diagonal,
    so the numerator is one fused scalar_tensor_tensor(+row-accum) per pair.
  - denominator rides free on the ACT exp pass via activation(accum_out=...)
    except on the special pairs, which use a masked scalar_tensor_tensor on
    the otherwise-idle GPSIMD engine.
  - all post-matmul ops process PAIRS of 512-col psum banks (FD=1024) to
    amortize per-instruction overheads.
"""

import numpy as np
import ml_dtypes

BF16 = ml_dtypes.bfloat16

B = 4096          # batch (rows of f1/f2)
D = 1024          # feature dim
N = 2 * B         # total rows
NCORES = 8
RPC = N // NCORES  # rows per core = 1024
P = 128           # partitions
KC = D // P       # K chunks = 8
MBLK = RPC // P   # row blocks per core = 8
T_INV = 20.0      # 1/t
EPS_W = 1e-5

# column spans (per core, in permuted column space). Each span is processed
# as pairs of 512-wide matmul tiles. The first span of each half holds every
# row-block's special (masked) columns.
SPANS = [1024, 1024, 2048, 1024, 1024, 2048]
SPAN_OFF = [0, 1024, 2048, 4096, 5120, 6144]
SPECIAL_SPANS = (0, 3)

# matmul input format: fp8 e4m3 with DoubleRow (2 fp8 weights/cell, 0.5
# cycles/row) — features scaled by FP8_SCALE to stay clear of subnormals;
# the similarity comes back scaled by FP8_SCALE^2, folded into the ACT scale.
USE_FP8 = True
FP8_SCALE = 16.0
MM_SCALE = T_INV / (FP8_SCALE * FP8_SCALE) if USE_FP8 else T_INV
IN_NP_DT = ml_dtypes.float8_e4m3fn if USE_FP8 else BF16

_NC_CACHE = {}


def _split_sync_waits(nc):
    """Legalize for this walrus build: TPB instruction structs hold only ONE
    inline sync-wait (EventSemaphore: two), so move excess waits onto
    standalone EventSemaphore (wait-only) instructions placed just before, on
    the same engine. Engine sequencers are in-order, so semantics are
    preserved."""
    import concourse.mybir as mybir

    n_new = 0
    for f in nc.m.functions:
        for b in f.blocks:
            out = []
            changed = False
            for inst in b.instructions:
                si = getattr(inst, "sync_info", None)
                waits = list(si.on_wait) if si and si.on_wait else []
                if len(waits) > 1:
                    excess, keep = waits[:-1], waits[-1:]
                    for i in range(0, len(excess), 2):
                        ev = mybir.InstEventSemaphore(
                            name=f"Wsplit-{n_new}", ins=[], outs=[])
                        ev.engine = inst.engine
                        ev.sync_info = mybir.SyncInfo(
                            on_wait=excess[i:i + 2], on_update=[])
                        out.append(ev)
                        n_new += 1
                    inst.sync_info = mybir.SyncInfo(
                        on_wait=keep, on_update=list(si.on_update))
                    changed = True
                out.append(inst)
            if changed:
                b.instructions = out
    return n_new


def _build_nc():
    """Build the single SPMD Bass program (identical on all cores)."""
    import concourse.bass as bass
    import concourse.tile as tile
    import concourse.mybir as mybir

    f32 = mybir.dt.float32
    bf16 = mybir.dt.bfloat16
    mm_dt = mybir.dt.float8e4 if USE_FP8 else bf16
    GS = 2 if USE_FP8 else 1      # k-chunks per matmul (DoubleRow pairs)
    NG = KC // GS
    perf_mode = mybir.MatmulPerfMode.DoubleRow if USE_FP8 else None
    MUL = mybir.AluOpType.mult
    ADD = mybir.AluOpType.add
    EXP = mybir.ActivationFunctionType.Exp
    AXX = mybir.AxisListType.X

    nc = bass.Bass(trn_type="TRN2", debug=False)

    rhs_d = nc.dram_tensor("rhs", [KC, P, N], mm_dt, kind="ExternalInput")
    lhsT_d = nc.dram_tensor("lhsT", [KC, P, RPC], mm_dt, kind="ExternalInput")
    w_d = nc.dram_tensor("w", [RPC, B], bf16, kind="ExternalInput")
    masks_d = nc.dram_tensor("masks", [MBLK, P, 1024], bf16,
                             kind="ExternalInput")
    num_d = nc.dram_tensor("num_out", [P, MBLK], f32, kind="ExternalOutput")
    den_d = nc.dram_tensor("den_out", [P, MBLK], f32, kind="ExternalOutput")

    with tile.TileContext(nc) as tc:
        with (
            tc.tile_pool(name="const", bufs=1) as cpool,
            tc.tile_pool(name="rhsp", bufs=2) as rpool,
            tc.tile_pool(name="ep", bufs=8) as epool,
            tc.tile_pool(name="scrp", bufs=4) as spool,
            tc.tile_pool(name="accp", bufs=1) as apool,
            tc.tile_pool(name="psum", bufs=4, space="PSUM") as pspool,
        ):
            # per-k-group lhsT tiles: exact deps so matmuls start as soon as
            # the first k-group lands instead of waiting for the whole thing
            lhsT_sb = [cpool.tile([P, GS, RPC], mm_dt, name=f"lhsT{g}")
                       for g in range(NG)]
            for g in range(NG):
                nc.sync.dma_start(
                    lhsT_sb[g][:],
                    lhsT_d[g * GS:(g + 1) * GS].rearrange("k p m -> p k m"))
            masks_sb = cpool.tile([P, MBLK, 1024], bf16)
            w_sb = [cpool.tile([P, B], bf16, name=f"w{m}") for m in range(MBLK)]
            w_r = w_d[:].rearrange("(m p) c -> m p c", p=P)

            NSLOT = 8  # pairs per row-block
            num_parts = apool.tile([P, MBLK, NSLOT], f32)
            den_parts = apool.tile([P, MBLK, NSLOT], f32)
            num_fin = apool.tile([P, MBLK], f32)
            den_fin = apool.tile([P, MBLK], f32)

            slot_base = 0
            for s, (span, off) in enumerate(zip(SPANS, SPAN_OFF)):
                npairs = span // 1024
                rhs_sb = [rpool.tile([P, GS, 2048], mm_dt, tag=f"rhs{g}",
                                     name=f"rhs{g}_{s}")
                          for g in range(NG)]
                for g in range(NG):
                    nc.sync.dma_start(
                        rhs_sb[g][:, :, :span],
                        rhs_d[g * GS:(g + 1) * GS, :, off:off + span]
                        .rearrange("k p n -> p k n")
                    )
                if s == 0:
                    # bulk weight data rides behind the first rhs span: the
                    # PE never waits on it, only the (laggier) DVE
                    nc.sync.dma_start(
                        masks_sb[:], masks_d[:].rearrange("j p f -> p j f"))
                    for m in range(MBLK):
                        nc.sync.dma_start(w_sb[m][:], w_r[m])
                half_off = off % B  # column offset within the w matrix
                for m in range(MBLK):
                    for pr in range(npairs):
                        ps = pspool.tile([P, 1024], f32, tag="ps")
                        for h in range(2):  # two 512-wide matmul groups
                            for g in range(NG):
                                nc.tensor.matmul(
                                    ps[:, h * 512:(h + 1) * 512],
                                    lhsT_sb[g][:, :, m * P:(m + 1) * P],
                                    rhs_sb[g][:, :,
                                              pr * 1024 + h * 512:
                                              pr * 1024 + (h + 1) * 512],
                                    start=(g == 0),
                                    stop=(g == NG - 1),
                                    perf_mode=perf_mode,
                                )
                        e = epool.tile([P, 1024], bf16, tag="e")
                        slot = slot_base + pr
                        if s in SPECIAL_SPANS:
                            # holds this row-block's excluded columns:
                            # masked denominator on DVE
                            nc.scalar.activation(e[:], ps[:], EXP,
                                                 scale=MM_SCALE)
                            scr = spool.tile([P, 1024], bf16, tag="gscr")
                            nc.vector.scalar_tensor_tensor(
                                out=scr[:],
                                in0=e[:],
                                scalar=1.0,
                                in1=masks_sb[:, m],
                                op0=MUL,
                                op1=MUL,
                                accum_out=den_parts[:, m, slot:slot + 1],
                            )
                        else:
                            nc.scalar.activation(
                                e[:], ps[:], EXP, scale=MM_SCALE,
                                accum_out=den_parts[:, m, slot:slot + 1],
                            )
                        # numerator: sum(e * w) over this pair's columns
                        wc0 = half_off + pr * 1024
                        scr2 = spool.tile([P, 1024], bf16, tag="scr")
                        nc.vector.scalar_tensor_tensor(
                            out=scr2[:],
                            in0=e[:],
                            scalar=1.0,
                            in1=w_sb[m][:, wc0:wc0 + 1024],
                            op0=MUL,
                            op1=MUL,
                            accum_out=num_parts[:, m, slot:slot + 1],
                        )
                slot_base += npairs

            for m in range(MBLK):
                nc.vector.tensor_reduce(
                    num_fin[:, m:m + 1], num_parts[:, m, :], axis=AXX, op=ADD
                )
                nc.vector.tensor_reduce(
                    den_fin[:, m:m + 1], den_parts[:, m, :], axis=AXX, op=ADD
                )
            nc.sync.dma_start(num_d[:], num_fin[:])
            nc.sync.dma_start(den_d[:], den_fin[:])

    _split_sync_waits(nc)
    return nc


def get_nc():
    if "nc" not in _NC_CACHE:
        _NC_CACHE["nc"] = _build_nc()
    return _NC_CACHE["nc"]


def _make_masks():
    """masks[m, p, f] = 0 where f == 512*(m//4) + 128*(m%4) + p else 1."""
    mk = np.ones((MBLK, P, 1024), dtype=np.float32)
    for m in range(MBLK):
        pp = np.arange(P)
        mk[m, pp, 512 * (m // 4) + 128 * (m % 4) + pp] = 0.0
    return mk.astype(BF16)


def prep_inputs(feature1, feature2, S_weight, pre_label):
    """Build the 8 per-core input maps + host-side special terms."""
    f1 = np.ascontiguousarray(np.asarray(feature1, dtype=np.float32))
    f2 = np.ascontiguousarray(np.asarray(feature2, dtype=np.float32))
    S = np.asarray(S_weight, dtype=np.float32)
    labels = np.asarray(pre_label).astype(np.int64)

    H = np.concatenate([f1, f2], axis=0)            # [N, D] f32
    HT = np.ascontiguousarray(H.T)                  # [D, N] f32
    if USE_FP8:
        HT_bf = (HT * np.float32(FP8_SCALE)).astype(IN_NP_DT)
    else:
        HT_bf = HT.astype(IN_NP_DT)                 # [D, N]
    masks = _make_masks()

    in_maps = []
    for k in range(NCORES):
        R = RPC * k
        rho = R % B
        perm = (np.arange(B) + rho) % B             # per-half column rotation
        cols = np.concatenate([perm, B + perm])     # [N]
        rhs = np.ascontiguousarray(
            HT_bf[:, cols].reshape(KC, P, N))       # [8, 128, 8192]
        lhsT = np.ascontiguousarray(
            HT_bf[:, R:R + RPC].reshape(KC, P, RPC))  # [8, 128, 1024]
        rows = np.arange(rho, rho + RPC)
        Sp = S[rows][:, perm] + np.float32(EPS_W)   # [1024, 4096]
        cat = labels[rows][:, None] == labels[perm][None, :]
        w = np.where(cat, Sp, np.float32(0.0))
        ii = np.arange(RPC)
        w[ii, ii] = 0.0                             # special cols excluded
        in_maps.append({
            "rhs": rhs,
            "lhsT": lhsT,
            "w": np.ascontiguousarray(w.astype(BF16)),
            "masks": masks,
        })

    # host-side special terms in float64
    H64 = H.astype(np.float64)
    sim_ii = np.einsum("ij,ij->i", H64, H64)            # [N] ~ 1.0
    cross = np.einsum("ij,ij->i", f1.astype(np.float64),
                      f2.astype(np.float64))            # [B]
    e_ii = np.exp(sim_ii * T_INV)
    e_cross = np.exp(np.concatenate([cross, cross]) * T_INV)
    s_ii = S.diagonal().astype(np.float64)
    s_cross = np.concatenate([s_ii, s_ii])
    num_special = EPS_W * e_ii + (s_cross + EPS_W) * e_cross  # [N]
    return in_maps, num_special


def postprocess(results, num_special):
    num = np.concatenate(
        [np.asarray(r["num_out"]).T.reshape(-1) for r in results])
    den = np.concatenate(
        [np.asarray(r["den_out"]).T.reshape(-1) for r in results])
    num_total = num.astype(np.float64) + num_special
    loss = -np.mean(np.log(num_total / den.astype(np.float64)))
    return np.float32(loss)


def kernel(feature1, feature2, S_weight, pre_label):
    from concourse.bass_utils import run_bass_kernel_spmd

    nc = get_nc()
    in_maps, num_special = prep_inputs(feature1, feature2, S_weight, pre_label)
    res = run_bass_kernel_spmd(nc, in_maps, core_ids=list(range(NCORES)))
    return postprocess(res.results, num_special)


# revision 30
# speedup vs baseline: 1.1323x; 1.1323x over previous
"""Trainium2 Bass kernel for nn_Attention_loss_919123001759.

Contrastive-style loss:
    H = concat(f1, f2)               [N=8192, D=1024], rows L2-normalized
    e = exp(H @ H.T / t)             [N, N], t = 0.05
    num_i = sum_j e_ij * (S2_ij + eps) * cat_ij
    den_i = sum_j e_ij * negmask_ij          (negmask excludes j=i, |i-j|=B)
    loss = -mean(log(num_i / den_i))

Distribution: rows of H sharded across 8 NeuronCores (1024 rows each). Each
core computes its [1024, 8192] similarity block against the full H^T, fused
exp + masked row-reductions, and outputs per-row partial numerator /
denominator sums. The host adds the analytically-known "special" terms
(diagonal j=i and positive pair j=i+-B, computed in float64 from the raw
features) and performs the final log/mean.

Key tricks:
  - fp8(e4m3) DoubleRow matmul (0.5 cycles/row, ~150 TF/s measured). Only
    the special columns need more precision than fp8 gives, and those are
    excluded on-chip and added host-side in float64. Bulk |sim| <~ 0.2, and
    quantization bias cancels between numerator and denominator.
  - per-core column rotation (host-side gather) so the special diagonal
    lands at compile-time-known tiles -> one SPMD program for all 8 cores.
  - w = (S+eps)*cat precomputed on host (bf16) with zeroed special diagonal,
    so the numerator is one fused scalar_tensor_tensor(+row-accum) per pair.
  - denominator rides free on the ACT exp pass via activation(accum_out=...)
    except on the special pairs, which use a masked scalar_tensor_tensor on
    the otherwise-idle GPSIMD engine.
  - all post-matmul ops process PAIRS of 512-col psum banks (FD=1024) to
    amortize per-instruction overheads.
"""

import numpy as np
import ml_dtypes

BF16 = ml_dtypes.bfloat16

B = 4096          # batch (rows of f1/f2)
D = 1024          # feature dim
N = 2 * B         # total rows
NCORES = 8
RPC = N // NCORES  # rows per core = 1024
P = 128           # partitions
KC = D // P       # K chunks = 8
MBLK = RPC // P   # row blocks per core = 8
T_INV = 20.0      # 1/t
EPS_W = 1e-5

# column spans (per core, in permuted column space). Each span is processed
# as pairs of 512-wide matmul tiles. The first span of each half holds every
# row-block's special (masked) columns.
SPANS = [1024, 1024, 2048, 1024, 1024, 2048]
SPAN_OFF = [0, 1024, 2048, 4096, 5120, 6144]
SPECIAL_SPANS = (0, 3)

# matmul input format: fp8 e4m3 with DoubleRow (2 fp8 weights/cell, 0.5
# cycles/row) — features scaled by FP8_SCALE to stay clear of subnormals;
# the similarity comes back scaled by FP8_SCALE^2, folded into the ACT scale.
USE_FP8 = True
FP8_SCALE = 16.0
MM_SCALE = T_INV / (FP8_SCALE * FP8_SCALE) if USE_FP8 else T_INV
IN_NP_DT = ml_dtypes.float8_e4m3fn if USE_FP8 else BF16

_NC_CACHE = {}


def _split_sync_waits(nc):
    """Legalize for this walrus build: TPB instruction structs hold only ONE
    inline sync-wait (EventSemaphore: two), so move excess waits onto
    standalone EventSemaphore (wait-only) instructions placed just before, on
    the same engine. Engine sequencers are in-order, so semantics are
    preserved."""
    import concourse.mybir as mybir

    n_new = 0
    for f in nc.m.functions:
        for b in f.blocks:
            out = []
            changed = False
            for inst in b.instructions:
                si = getattr(inst, "sync_info", None)
                waits = list(si.on_wait) if si and si.on_wait else []
                if len(waits) > 1:
                    excess, keep = waits[:-1], waits[-1:]
                    for i in range(0, len(excess), 2):
                        ev = mybir.InstEventSemaphore(
                            name=f"Wsplit-{n_new}", ins=[], outs=[])
                        ev.engine = inst.engine
                        ev.sync_info = mybir.SyncInfo(
                            on_wait=excess[i:i + 2], on_update=[])
                        out.append(ev)
                        n_new += 1
                    inst.sync_info = mybir.SyncInfo(
                        on_wait=keep, on_update=list(si.on_update))
                    changed = True
                out.append(inst)
            if changed:
                b.instructions = out
    return n_new


def _build_nc():
    """Build the single SPMD Bass program (identical on all cores)."""
    import concourse.bass as bass
    import concourse.tile as tile
    import concourse.mybir as mybir

    f32 = mybir.dt.float32
    bf16 = mybir.dt.bfloat16
    mm_dt = mybir.dt.float8e4 if USE_FP8 else bf16
    GS = 2 if USE_FP8 else 1      # k-chunks per matmul (DoubleRow pairs)
    NG = KC // GS
    perf_mode = mybir.MatmulPerfMode.DoubleRow if USE_FP8 else None
    MUL = mybir.AluOpType.mult
    ADD = mybir.AluOpType.add
    EXP = mybir.ActivationFunctionType.Exp
    AXX = mybir.AxisListType.X

    nc = bass.Bass(trn_type="TRN2", debug=False)

    rhs_d = nc.dram_tensor("rhs", [KC, P, N], mm_dt, kind="ExternalInput")
    lhsT_d = nc.dram_tensor("lhsT", [KC, P, RPC], mm_dt, kind="ExternalInput")
    w_dt = mybir.dt.float8e4 if USE_FP8 else bf16
    w_d = nc.dram_tensor("w", [RPC, B], w_dt, kind="ExternalInput")
    # sliding mask strip: masks[p, y] = 0 iff y == 1024 + p; slicing at
    # offset 1024 - 128*m yields the per-row-block diagonal mask
    masks_d = nc.dram_tensor("masks", [P, 2048], bf16, kind="ExternalInput")
    num_d = nc.dram_tensor("num_out", [P, MBLK], f32, kind="ExternalOutput")
    den_d = nc.dram_tensor("den_out", [P, MBLK], f32, kind="ExternalOutput")

    with tile.TileContext(nc) as tc:
        with (
            tc.tile_pool(name="const", bufs=1) as cpool,
            tc.tile_pool(name="rhsp", bufs=2) as rpool,
            tc.tile_pool(name="ep", bufs=10) as epool,
            tc.tile_pool(name="scrp", bufs=4) as spool,
            tc.tile_pool(name="accp", bufs=1) as apool,
            tc.tile_pool(name="psum", bufs=4, space="PSUM") as pspool,
        ):
            # per-k-group lhsT tiles: exact deps so matmuls start as soon as
            # the first k-group lands instead of waiting for the whole thing
            lhsT_sb = [cpool.tile([P, GS, RPC], mm_dt, name=f"lhsT{g}")
                       for g in range(NG)]
            for g in range(NG):
                nc.sync.dma_start(
                    lhsT_sb[g][:],
                    lhsT_d[g * GS:(g + 1) * GS].rearrange("k p m -> p k m"))
            masks_sb = cpool.tile([P, 2048], bf16)
            w_sb = [cpool.tile([P, B], w_dt, name=f"w{m}")
                    for m in range(MBLK)]
            w_r = w_d[:].rearrange("(m p) c -> m p c", p=P)

            NSLOT = 8  # pairs per row-block
            num_parts = apool.tile([P, MBLK, NSLOT], f32)
            den_parts = apool.tile([P, MBLK, NSLOT], f32)
            num_fin = apool.tile([P, MBLK], f32)
            den_fin = apool.tile([P, MBLK], f32)

            slot_base = 0
            for s, (span, off) in enumerate(zip(SPANS, SPAN_OFF)):
                npairs = span // 1024
                rhs_sb = [rpool.tile([P, GS, 2048], mm_dt, tag=f"rhs{g}",
                                     name=f"rhs{g}_{s}")
                          for g in range(NG)]
                for g in range(NG):
                    nc.sync.dma_start(
                        rhs_sb[g][:, :, :span],
                        rhs_d[g * GS:(g + 1) * GS, :, off:off + span]
                        .rearrange("k p n -> p k n")
                    )
                # weight data (small in fp8) rides behind the first rhs span;
                # span 0's DVE ops read every w tile, so all must precede the
                # first compute in program order
                if s == 0:
                    nc.sync.dma_start(masks_sb[:], masks_d[:])
                    for m in range(MBLK):
                        nc.sync.dma_start(w_sb[m][:], w_r[m])
                half_off = off % B  # column offset within the w matrix
                for m in range(MBLK):
                    for pr in range(npairs):
                        ps = pspool.tile([P, 1024], f32, tag="ps")
                        for h in range(2):  # two 512-wide matmul groups
                            for g in range(NG):
                                nc.tensor.matmul(
                                    ps[:, h * 512:(h + 1) * 512],
                                    lhsT_sb[g][:, :, m * P:(m + 1) * P],
                                    rhs_sb[g][:, :,
                                              pr * 1024 + h * 512:
                                              pr * 1024 + (h + 1) * 512],
                                    start=(g == 0),
                                    stop=(g == NG - 1),
                                    perf_mode=perf_mode,
                                )
                        e = epool.tile([P, 1024], bf16, tag="e")
                        slot = slot_base + pr
                        if s in SPECIAL_SPANS:
                            # holds this row-block's excluded columns:
                            # masked denominator on DVE
                            nc.scalar.activation(e[:], ps[:], EXP,
                                                 scale=MM_SCALE)
                            scr = spool.tile([P, 1024], bf16, tag="gscr")
                            o_m = 1024 - 128 * m
                            nc.vector.scalar_tensor_tensor(
                                out=scr[:],
                                in0=e[:],
                                scalar=1.0,
                                in1=masks_sb[:, o_m:o_m + 1024],
                                op0=MUL,
                                op1=MUL,
                                accum_out=den_parts[:, m, slot:slot + 1],
                            )
                        else:
                            nc.scalar.activation(
                                e[:], ps[:], EXP, scale=MM_SCALE,
                                accum_out=den_parts[:, m, slot:slot + 1],
                            )
                        # numerator: sum(e * w) over this pair's columns
                        wc0 = half_off + pr * 1024
                        scr2 = spool.tile([P, 1024], bf16, tag="scr")
                        nc.vector.scalar_tensor_tensor(
                            out=scr2[:],
                            in0=e[:],
                            scalar=1.0,
                            in1=w_sb[m][:, wc0:wc0 + 1024],
                            op0=MUL,
                            op1=MUL,
                            accum_out=num_parts[:, m, slot:slot + 1],
                        )
                slot_base += npairs

            for m in range(MBLK):
                nc.vector.tensor_reduce(
                    num_fin[:, m:m + 1], num_parts[:, m, :], axis=AXX, op=ADD
                )
                nc.vector.tensor_reduce(
                    den_fin[:, m:m + 1], den_parts[:, m, :], axis=AXX, op=ADD
                )
            nc.sync.dma_start(num_d[:], num_fin[:])
            nc.sync.dma_start(den_d[:], den_fin[:])

    _split_sync_waits(nc)
    return nc


def get_nc():
    if "nc" not in _NC_CACHE:
        _NC_CACHE["nc"] = _build_nc()
    return _NC_CACHE["nc"]


def _make_masks():
    """Sliding strip: masks[p, y] = 0 where y == 1024 + p else 1."""
    mk = np.ones((P, 2048), dtype=np.float32)
    pp = np.arange(P)
    mk[pp, 1024 + pp] = 0.0
    return mk.astype(BF16)


def prep_inputs(feature1, feature2, S_weight, pre_label):
    """Build the 8 per-core input maps + host-side special terms."""
    f1 = np.ascontiguousarray(np.asarray(feature1, dtype=np.float32))
    f2 = np.ascontiguousarray(np.asarray(feature2, dtype=np.float32))
    S = np.asarray(S_weight, dtype=np.float32)
    labels = np.asarray(pre_label).astype(np.int64)

    H = np.concatenate([f1, f2], axis=0)            # [N, D] f32
    HT = np.ascontiguousarray(H.T)                  # [D, N] f32
    if USE_FP8:
        HT_bf = (HT * np.float32(FP8_SCALE)).astype(IN_NP_DT)
    else:
        HT_bf = HT.astype(IN_NP_DT)                 # [D, N]
    masks = _make_masks()

    in_maps = []
    for k in range(NCORES):
        R = RPC * k
        rho = R % B
        perm = (np.arange(B) + rho) % B             # per-half column rotation
        cols = np.concatenate([perm, B + perm])     # [N]
        rhs = np.ascontiguousarray(
            HT_bf[:, cols].reshape(KC, P, N))       # [8, 128, 8192]
        lhsT = np.ascontiguousarray(
            HT_bf[:, R:R + RPC].reshape(KC, P, RPC))  # [8, 128, 1024]
        rows = np.arange(rho, rho + RPC)
        Sp = S[rows][:, perm] + np.float32(EPS_W)   # [1024, 4096]
        cat = labels[rows][:, None] == labels[perm][None, :]
        w = np.where(cat, Sp, np.float32(0.0))
        ii = np.arange(RPC)
        w[ii, ii] = 0.0                             # special cols excluded
        w_dt = IN_NP_DT if USE_FP8 else BF16
        in_maps.append({
            "rhs": rhs,
            "lhsT": lhsT,
            "w": np.ascontiguousarray(w.astype(w_dt)),
            "masks": masks,
        })

    # host-side special terms in float64
    H64 = H.astype(np.float64)
    sim_ii = np.einsum("ij,ij->i", H64, H64)            # [N] ~ 1.0
    cross = np.einsum("ij,ij->i", f1.astype(np.float64),
                      f2.astype(np.float64))            # [B]
    e_ii = np.exp(sim_ii * T_INV)
    e_cross = np.exp(np.concatenate([cross, cross]) * T_INV)
    s_ii = S.diagonal().astype(np.float64)
    s_cross = np.concatenate([s_ii, s_ii])
    num_special = EPS_W * e_ii + (s_cross + EPS_W) * e_cross  # [N]
    return in_maps, num_special


def postprocess(results, num_special):
    num = np.concatenate(
        [np.asarray(r["num_out"]).T.reshape(-1) for r in results])
    den = np.concatenate(
        [np.asarray(r["den_out"]).T.reshape(-1) for r in results])
    num_total = num.astype(np.float64) + num_special
    loss = -np.mean(np.log(num_total / den.astype(np.float64)))
    return np.float32(loss)


def kernel(feature1, feature2, S_weight, pre_label):
    from concourse.bass_utils import run_bass_kernel_spmd

    nc = get_nc()
    in_maps, num_special = prep_inputs(feature1, feature2, S_weight, pre_label)
    res = run_bass_kernel_spmd(nc, in_maps, core_ids=list(range(NCORES)))
    return postprocess(res.results, num_special)


# revision 36
# speedup vs baseline: 1.1416x; 1.0082x over previous
"""Trainium2 Bass kernel for nn_Attention_loss_919123001759.

Contrastive-style loss:
    H = concat(f1, f2)               [N=8192, D=1024], rows L2-normalized
    e = exp(H @ H.T / t)             [N, N], t = 0.05
    num_i = sum_j e_ij * (S2_ij + eps) * cat_ij
    den_i = sum_j e_ij * negmask_ij          (negmask excludes j=i, |i-j|=B)
    loss = -mean(log(num_i / den_i))

Distribution: rows of H sharded across 8 NeuronCores (1024 rows each). Each
core computes its [1024, 8192] similarity block against the full H^T, fused
exp + masked row-reductions, and outputs per-row partial numerator /
denominator sums. The host adds the analytically-known "special" terms
(diagonal j=i and positive pair j=i+-B, computed in float64 from the raw
features) and performs the final log/mean.

Key tricks:
  - fp8(e4m3) DoubleRow matmul (0.5 cycles/row, ~150 TF/s measured). Only
    the special columns need more precision than fp8 gives, and those are
    excluded on-chip and added host-side in float64. Bulk |sim| <~ 0.2, and
    quantization bias cancels between numerator and denominator.
  - per-core column rotation (host-side gather) so the special diagonal
    lands at compile-time-known tiles -> one SPMD program for all 8 cores.
  - w = (S+eps)*cat precomputed on host (bf16) with zeroed special diagonal,
    so the numerator is one fused scalar_tensor_tensor(+row-accum) per pair.
  - denominator rides free on the ACT exp pass via activation(accum_out=...)
    except on the special pairs, which use a masked scalar_tensor_tensor on
    the otherwise-idle GPSIMD engine.
  - all post-matmul ops process PAIRS of 512-col psum banks (FD=1024) to
    amortize per-instruction overheads.
"""

import numpy as np
import ml_dtypes

BF16 = ml_dtypes.bfloat16

B = 4096          # batch (rows of f1/f2)
D = 1024          # feature dim
N = 2 * B         # total rows
NCORES = 8
RPC = N // NCORES  # rows per core = 1024
P = 128           # partitions
KC = D // P       # K chunks = 8
MBLK = RPC // P   # row blocks per core = 8
T_INV = 20.0      # 1/t
EPS_W = 1e-5

# column spans (per core, in permuted column space). Each span is processed
# as pairs of 512-wide matmul tiles. The first span of each half holds every
# row-block's special (masked) columns.
SPANS = [1024, 1024, 2048, 1024, 1024, 2048]
SPAN_OFF = [0, 1024, 2048, 4096, 5120, 6144]
SPECIAL_SPANS = (0, 3)

# matmul input format: fp8 e4m3 with DoubleRow (2 fp8 weights/cell, 0.5
# cycles/row) — features scaled by FP8_SCALE to stay clear of subnormals;
# the similarity comes back scaled by FP8_SCALE^2, folded into the ACT scale.
USE_FP8 = True
FP8_SCALE = 16.0
MM_SCALE = T_INV / (FP8_SCALE * FP8_SCALE) if USE_FP8 else T_INV
IN_NP_DT = ml_dtypes.float8_e4m3fn if USE_FP8 else BF16

_NC_CACHE = {}


def _split_sync_waits(nc):
    """Legalize for this walrus build: TPB instruction structs hold only ONE
    inline sync-wait (EventSemaphore: two), so move excess waits onto
    standalone EventSemaphore (wait-only) instructions placed just before, on
    the same engine. Engine sequencers are in-order, so semantics are
    preserved."""
    import concourse.mybir as mybir

    n_new = 0
    for f in nc.m.functions:
        for b in f.blocks:
            out = []
            changed = False
            for inst in b.instructions:
                si = getattr(inst, "sync_info", None)
                waits = list(si.on_wait) if si and si.on_wait else []
                if len(waits) > 1:
                    excess, keep = waits[:-1], waits[-1:]
                    for i in range(0, len(excess), 2):
                        ev = mybir.InstEventSemaphore(
                            name=f"Wsplit-{n_new}", ins=[], outs=[])
                        ev.engine = inst.engine
                        ev.sync_info = mybir.SyncInfo(
                            on_wait=excess[i:i + 2], on_update=[])
                        out.append(ev)
                        n_new += 1
                    inst.sync_info = mybir.SyncInfo(
                        on_wait=keep, on_update=list(si.on_update))
                    changed = True
                out.append(inst)
            if changed:
                b.instructions = out
    return n_new


def _build_nc():
    """Build the single SPMD Bass program (identical on all cores)."""
    import concourse.bass as bass
    import concourse.tile as tile
    import concourse.mybir as mybir

    f32 = mybir.dt.float32
    bf16 = mybir.dt.bfloat16
    mm_dt = mybir.dt.float8e4 if USE_FP8 else bf16
    GS = 2 if USE_FP8 else 1      # k-chunks per matmul (DoubleRow pairs)
    NG = KC // GS
    perf_mode = mybir.MatmulPerfMode.DoubleRow if USE_FP8 else None
    MUL = mybir.AluOpType.mult
    ADD = mybir.AluOpType.add
    EXP = mybir.ActivationFunctionType.Exp
    AXX = mybir.AxisListType.X

    nc = bass.Bass(trn_type="TRN2", debug=False)

    rhs_d = nc.dram_tensor("rhs", [KC, P, N], mm_dt, kind="ExternalInput")
    lhsT_d = nc.dram_tensor("lhsT", [KC, P, RPC], mm_dt, kind="ExternalInput")
    w_dt = mybir.dt.float8e4 if USE_FP8 else bf16
    w_d = nc.dram_tensor("w", [RPC, B], w_dt, kind="ExternalInput")
    # sliding mask strip: masks[p, y] = 0 iff y == 1024 + p; slicing at
    # offset 1024 - 128*m yields the per-row-block diagonal mask
    masks_d = nc.dram_tensor("masks", [P, 2048], bf16, kind="ExternalInput")
    num_d = nc.dram_tensor("num_out", [P, MBLK, 8], f32,
                           kind="ExternalOutput")
    den_d = nc.dram_tensor("den_out", [P, MBLK, 8], f32,
                           kind="ExternalOutput")

    with tile.TileContext(nc) as tc:
        with (
            tc.tile_pool(name="const", bufs=1) as cpool,
            tc.tile_pool(name="rhsp", bufs=2) as rpool,
            tc.tile_pool(name="ep", bufs=10) as epool,
            tc.tile_pool(name="scrp", bufs=4) as spool,
            tc.tile_pool(name="accp", bufs=1) as apool,
            tc.tile_pool(name="psum", bufs=4, space="PSUM") as pspool,
        ):
            # per-k-group lhsT tiles: exact deps so matmuls start as soon as
            # the first k-group lands instead of waiting for the whole thing
            lhsT_sb = [cpool.tile([P, GS, RPC], mm_dt, name=f"lhsT{g}")
                       for g in range(NG)]
            for g in range(NG):
                nc.sync.dma_start(
                    lhsT_sb[g][:],
                    lhsT_d[g * GS:(g + 1) * GS].rearrange("k p m -> p k m"))
            masks_sb = cpool.tile([P, 2048], bf16)
            w_sb = [cpool.tile([P, B], w_dt, name=f"w{m}")
                    for m in range(MBLK)]
            w_r = w_d[:].rearrange("(m p) c -> m p c", p=P)

            NSLOT = 8  # pairs per row-block
            num_parts = apool.tile([P, MBLK, NSLOT], f32)
            den_parts = apool.tile([P, MBLK, NSLOT], f32)

            slot_base = 0
            for s, (span, off) in enumerate(zip(SPANS, SPAN_OFF)):
                npairs = span // 1024

                def load_span(si):
                    tiles = [rpool.tile([P, GS, 2048], mm_dt, tag=f"rhs{g}",
                                        name=f"rhs{g}_{si}")
                             for g in range(NG)]
                    for g in range(NG):
                        nc.sync.dma_start(
                            tiles[g][:, :, :SPANS[si]],
                            rhs_d[g * GS:(g + 1) * GS, :,
                                  SPAN_OFF[si]:SPAN_OFF[si] + SPANS[si]]
                            .rearrange("k p n -> p k n")
                        )
                    return tiles

                # weight data (small in fp8) rides behind the first rhs span;
                # span 0's DVE ops read every w tile, so all must precede the
                # first compute in program order. The second rhs span is
                # prefetched ahead of w so it keeps DMA-queue priority.
                if s == 0:
                    rhs_sb = load_span(0)
                    rhs_pf1 = load_span(1)
                    nc.sync.dma_start(masks_sb[:], masks_d[:])
                    for m in range(MBLK):
                        nc.sync.dma_start(w_sb[m][:], w_r[m])
                elif s == 1:
                    rhs_sb = rhs_pf1  # already prefetched
                else:
                    rhs_sb = load_span(s)
                half_off = off % B  # column offset within the w matrix
                for m in range(MBLK):
                    for pr in range(npairs):
                        ps = pspool.tile([P, 1024], f32, tag="ps")
                        for h in range(2):  # two 512-wide matmul groups
                            for g in range(NG):
                                nc.tensor.matmul(
                                    ps[:, h * 512:(h + 1) * 512],
                                    lhsT_sb[g][:, :, m * P:(m + 1) * P],
                                    rhs_sb[g][:, :,
                                              pr * 1024 + h * 512:
                                              pr * 1024 + (h + 1) * 512],
                                    start=(g == 0),
                                    stop=(g == NG - 1),
                                    perf_mode=perf_mode,
                                )
                        e = epool.tile([P, 1024], bf16, tag="e")
                        slot = slot_base + pr
                        if s in SPECIAL_SPANS:
                            # holds this row-block's excluded columns:
                            # masked denominator on DVE
                            nc.scalar.activation(e[:], ps[:], EXP,
                                                 scale=MM_SCALE)
                            scr = spool.tile([P, 1024], bf16, tag="gscr")
                            o_m = 1024 - 128 * m
                            nc.vector.scalar_tensor_tensor(
                                out=scr[:],
                                in0=e[:],
                                scalar=1.0,
                                in1=masks_sb[:, o_m:o_m + 1024],
                                op0=MUL,
                                op1=MUL,
                                accum_out=den_parts[:, m, slot:slot + 1],
                            )
                        else:
                            nc.scalar.activation(
                                e[:], ps[:], EXP, scale=MM_SCALE,
                                accum_out=den_parts[:, m, slot:slot + 1],
                            )
                        # numerator: sum(e * w) over this pair's columns
                        wc0 = half_off + pr * 1024
                        scr2 = spool.tile([P, 1024], bf16, tag="scr")
                        nc.vector.scalar_tensor_tensor(
                            out=scr2[:],
                            in0=e[:],
                            scalar=1.0,
                            in1=w_sb[m][:, wc0:wc0 + 1024],
                            op0=MUL,
                            op1=MUL,
                            accum_out=num_parts[:, m, slot:slot + 1],
                        )
                slot_base += npairs

            # final 8-way slot reduction happens on the host
            nc.sync.dma_start(num_d[:], num_parts[:])
            nc.sync.dma_start(den_d[:], den_parts[:])

    _split_sync_waits(nc)
    return nc


def get_nc():
    if "nc" not in _NC_CACHE:
        _NC_CACHE["nc"] = _build_nc()
    return _NC_CACHE["nc"]


def _make_masks():
    """Sliding strip: masks[p, y] = 0 where y == 1024 + p else 1."""
    mk = np.ones((P, 2048), dtype=np.float32)
    pp = np.arange(P)
    mk[pp, 1024 + pp] = 0.0
    return mk.astype(BF16)


def prep_inputs(feature1, feature2, S_weight, pre_label):
    """Build the 8 per-core input maps + host-side special terms."""
    f1 = np.ascontiguousarray(np.asarray(feature1, dtype=np.float32))
    f2 = np.ascontiguousarray(np.asarray(feature2, dtype=np.float32))
    S = np.asarray(S_weight, dtype=np.float32)
    labels = np.asarray(pre_label).astype(np.int64)

    H = np.concatenate([f1, f2], axis=0)            # [N, D] f32
    HT = np.ascontiguousarray(H.T)                  # [D, N] f32
    if USE_FP8:
        HT_bf = (HT * np.float32(FP8_SCALE)).astype(IN_NP_DT)
    else:
        HT_bf = HT.astype(IN_NP_DT)                 # [D, N]
    masks = _make_masks()

    in_maps = []
    for k in range(NCORES):
        R = RPC * k
        rho = R % B
        perm = (np.arange(B) + rho) % B             # per-half column rotation
        cols = np.concatenate([perm, B + perm])     # [N]
        rhs = np.ascontiguousarray(
            HT_bf[:, cols].reshape(KC, P, N))       # [8, 128, 8192]
        lhsT = np.ascontiguousarray(
            HT_bf[:, R:R + RPC].reshape(KC, P, RPC))  # [8, 128, 1024]
        rows = np.arange(rho, rho + RPC)
        Sp = S[rows][:, perm] + np.float32(EPS_W)   # [1024, 4096]
        cat = labels[rows][:, None] == labels[perm][None, :]
        w = np.where(cat, Sp, np.float32(0.0))
        ii = np.arange(RPC)
        w[ii, ii] = 0.0                             # special cols excluded
        w_dt = IN_NP_DT if USE_FP8 else BF16
        in_maps.append({
            "rhs": rhs,
            "lhsT": lhsT,
            "w": np.ascontiguousarray(w.astype(w_dt)),
            "masks": masks,
        })

    # host-side special terms in float64
    H64 = H.astype(np.float64)
    sim_ii = np.einsum("ij,ij->i", H64, H64)            # [N] ~ 1.0
    cross = np.einsum("ij,ij->i", f1.astype(np.float64),
                      f2.astype(np.float64))            # [B]
    e_ii = np.exp(sim_ii * T_INV)
    e_cross = np.exp(np.concatenate([cross, cross]) * T_INV)
    s_ii = S.diagonal().astype(np.float64)
    s_cross = np.concatenate([s_ii, s_ii])
    num_special = EPS_W * e_ii + (s_cross + EPS_W) * e_cross  # [N]
    return in_maps, num_special


def postprocess(results, num_special):
    num = np.concatenate(
        [np.asarray(r["num_out"], dtype=np.float64).sum(-1).T.reshape(-1)
         for r in results])
    den = np.concatenate(
        [np.asarray(r["den_out"], dtype=np.float64).sum(-1).T.reshape(-1)
         for r in results])
    num_total = num.astype(np.float64) + num_special
    loss = -np.mean(np.log(num_total / den.astype(np.float64)))
    return np.float32(loss)


def kernel(feature1, feature2, S_weight, pre_label):
    from concourse.bass_utils import run_bass_kernel_spmd

    nc = get_nc()
    in_maps, num_special = prep_inputs(feature1, feature2, S_weight, pre_label)
    res = run_bass_kernel_spmd(nc, in_maps, core_ids=list(range(NCORES)))
    return postprocess(res.results, num_special)


# revision 37
# speedup vs baseline: 1.1471x; 1.0048x over previous
"""Trainium2 Bass kernel for nn_Attention_loss_919123001759.

Contrastive-style loss:
    H = concat(f1, f2)               [N=8192, D=1024], rows L2-normalized
    e = exp(H @ H.T / t)             [N, N], t = 0.05
    num_i = sum_j e_ij * (S2_ij + eps) * cat_ij
    den_i = sum_j e_ij * negmask_ij          (negmask excludes j=i, |i-j|=B)
    loss = -mean(log(num_i / den_i))

Distribution: rows of H sharded across 8 NeuronCores (1024 rows each). Each
core computes its [1024, 8192] similarity block against the full H^T, fused
exp + masked row-reductions, and outputs per-row partial numerator /
denominator sums. The host adds the analytically-known "special" terms
(diagonal j=i and positive pair j=i+-B, computed in float64 from the raw
features) and performs the final log/mean.

Key tricks:
  - fp8(e4m3) DoubleRow matmul (0.5 cycles/row, ~150 TF/s measured). Only
    the special columns need more precision than fp8 gives, and those are
    excluded on-chip and added host-side in float64. Bulk |sim| <~ 0.2, and
    quantization bias cancels between numerator and denominator.
  - per-core column rotation (host-side gather) so the special diagonal
    lands at compile-time-known tiles -> one SPMD program for all 8 cores.
  - w = (S+eps)*cat precomputed on host (bf16) with zeroed special diagonal,
    so the numerator is one fused scalar_tensor_tensor(+row-accum) per pair.
  - denominator rides free on the ACT exp pass via activation(accum_out=...)
    except on the special pairs, which use a masked scalar_tensor_tensor on
    the otherwise-idle GPSIMD engine.
  - all post-matmul ops process PAIRS of 512-col psum banks (FD=1024) to
    amortize per-instruction overheads.
"""

import numpy as np
import ml_dtypes

BF16 = ml_dtypes.bfloat16

B = 4096          # batch (rows of f1/f2)
D = 1024          # feature dim
N = 2 * B         # total rows
NCORES = 8
RPC = N // NCORES  # rows per core = 1024
P = 128           # partitions
KC = D // P       # K chunks = 8
MBLK = RPC // P   # row blocks per core = 8
T_INV = 20.0      # 1/t
EPS_W = 1e-5

# column spans (per core, in permuted column space). Each span is processed
# as pairs of 512-wide matmul tiles. The first span of each half holds every
# row-block's special (masked) columns.
SPANS = [1024, 1024, 2048, 1024, 1024, 2048]
SPAN_OFF = [0, 1024, 2048, 4096, 5120, 6144]
SPECIAL_SPANS = (0, 3)

# matmul input format: fp8 e4m3 with DoubleRow (2 fp8 weights/cell, 0.5
# cycles/row) — features scaled by FP8_SCALE to stay clear of subnormals;
# the similarity comes back scaled by FP8_SCALE^2, folded into the ACT scale.
USE_FP8 = True
FP8_SCALE = 16.0
MM_SCALE = T_INV / (FP8_SCALE * FP8_SCALE) if USE_FP8 else T_INV
IN_NP_DT = ml_dtypes.float8_e4m3fn if USE_FP8 else BF16

_NC_CACHE = {}


def _split_sync_waits(nc):
    """Legalize for this walrus build: TPB instruction structs hold only ONE
    inline sync-wait (EventSemaphore: two), so move excess waits onto
    standalone EventSemaphore (wait-only) instructions placed just before, on
    the same engine. Engine sequencers are in-order, so semantics are
    preserved."""
    import concourse.mybir as mybir

    n_new = 0
    for f in nc.m.functions:
        for b in f.blocks:
            out = []
            changed = False
            for inst in b.instructions:
                si = getattr(inst, "sync_info", None)
                waits = list(si.on_wait) if si and si.on_wait else []
                if len(waits) > 1:
                    excess, keep = waits[:-1], waits[-1:]
                    for i in range(0, len(excess), 2):
                        ev = mybir.InstEventSemaphore(
                            name=f"Wsplit-{n_new}", ins=[], outs=[])
                        ev.engine = inst.engine
                        ev.sync_info = mybir.SyncInfo(
                            on_wait=excess[i:i + 2], on_update=[])
                        out.append(ev)
                        n_new += 1
                    inst.sync_info = mybir.SyncInfo(
                        on_wait=keep, on_update=list(si.on_update))
                    changed = True
                out.append(inst)
            if changed:
                b.instructions = out
    return n_new


def _build_nc():
    """Build the single SPMD Bass program (identical on all cores)."""
    import concourse.bass as bass
    import concourse.tile as tile
    import concourse.mybir as mybir

    f32 = mybir.dt.float32
    bf16 = mybir.dt.bfloat16
    mm_dt = mybir.dt.float8e4 if USE_FP8 else bf16
    GS = 2 if USE_FP8 else 1      # k-chunks per matmul (DoubleRow pairs)
    NG = KC // GS
    perf_mode = mybir.MatmulPerfMode.DoubleRow if USE_FP8 else None
    MUL = mybir.AluOpType.mult
    ADD = mybir.AluOpType.add
    EXP = mybir.ActivationFunctionType.Exp
    AXX = mybir.AxisListType.X

    nc = bass.Bass(trn_type="TRN2", debug=False)

    rhs_d = nc.dram_tensor("rhs", [KC, P, N], mm_dt, kind="ExternalInput")
    lhsT_d = nc.dram_tensor("lhsT", [KC, P, RPC], mm_dt, kind="ExternalInput")
    w_dt = mybir.dt.float8e4 if USE_FP8 else bf16
    w_d = nc.dram_tensor("w", [RPC, B], w_dt, kind="ExternalInput")
    # sliding mask strip: masks[p, y] = 0 iff y == 1024 + p; slicing at
    # offset 1024 - 128*m yields the per-row-block diagonal mask
    masks_d = nc.dram_tensor("masks", [P, 2048], bf16, kind="ExternalInput")
    num_d = nc.dram_tensor("num_out", [P, MBLK, 8], f32,
                           kind="ExternalOutput")
    den_d = nc.dram_tensor("den_out", [P, MBLK, 8], f32,
                           kind="ExternalOutput")

    with tile.TileContext(nc) as tc:
        with (
            tc.tile_pool(name="const", bufs=1) as cpool,
            tc.tile_pool(name="rhsp", bufs=2) as rpool,
            tc.tile_pool(name="ep", bufs=10) as epool,
            tc.tile_pool(name="scrp", bufs=4) as spool,
            tc.tile_pool(name="accp", bufs=1) as apool,
            tc.tile_pool(name="psum", bufs=4, space="PSUM") as pspool,
        ):
            # per-k-group lhsT tiles: exact deps so matmuls start as soon as
            # the first k-group lands instead of waiting for the whole thing
            lhsT_sb = [cpool.tile([P, GS, RPC], mm_dt, name=f"lhsT{g}")
                       for g in range(NG)]
            for g in range(NG):
                nc.sync.dma_start(
                    lhsT_sb[g][:],
                    lhsT_d[g * GS:(g + 1) * GS].rearrange("k p m -> p k m"))
            masks_sb = cpool.tile([P, 2048], bf16)
            w_sb = [cpool.tile([P, B], w_dt, name=f"w{m}")
                    for m in range(MBLK)]
            w_r = w_d[:].rearrange("(m p) c -> m p c", p=P)

            NSLOT = 8  # pairs per row-block
            num_parts = apool.tile([P, MBLK, NSLOT], f32)
            den_parts = apool.tile([P, MBLK, NSLOT], f32)

            slot_base = 0
            for s, (span, off) in enumerate(zip(SPANS, SPAN_OFF)):
                npairs = span // 1024

                def load_span(si):
                    tiles = [rpool.tile([P, GS, 2048], mm_dt, tag=f"rhs{g}",
                                        name=f"rhs{g}_{si}")
                             for g in range(NG)]
                    for g in range(NG):
                        nc.sync.dma_start(
                            tiles[g][:, :, :SPANS[si]],
                            rhs_d[g * GS:(g + 1) * GS, :,
                                  SPAN_OFF[si]:SPAN_OFF[si] + SPANS[si]]
                            .rearrange("k p n -> p k n")
                        )
                    return tiles

                # weight data (small in fp8) rides behind the first rhs span;
                # span 0's DVE ops read every w tile, so all must precede the
                # first compute in program order. The second rhs span is
                # prefetched ahead of w so it keeps DMA-queue priority.
                if s == 0:
                    rhs_sb = load_span(0)
                    rhs_pf1 = load_span(1)
                    nc.sync.dma_start(masks_sb[:], masks_d[:])
                    for m in range(MBLK):
                        nc.sync.dma_start(w_sb[m][:], w_r[m])
                elif s == 1:
                    rhs_sb = rhs_pf1  # already prefetched
                else:
                    rhs_sb = load_span(s)
                half_off = off % B  # column offset within the w matrix
                for m in range(MBLK):
                    # k-group outer so consecutive matmuls reuse the same
                    # stationary weights across all this row-block's tiles
                    pss = [pspool.tile([P, 1024], f32, tag="ps",
                                       name=f"ps_{s}_{m}_{q}")
                           for q in range(npairs)]
                    for g in range(NG):
                        for q in range(npairs):
                            for h in range(2):
                                nc.tensor.matmul(
                                    pss[q][:, h * 512:(h + 1) * 512],
                                    lhsT_sb[g][:, :, m * P:(m + 1) * P],
                                    rhs_sb[g][:, :,
                                              q * 1024 + h * 512:
                                              q * 1024 + (h + 1) * 512],
                                    start=(g == 0),
                                    stop=(g == NG - 1),
                                    perf_mode=perf_mode,
                                )
                    for pr in range(npairs):
                        ps = pss[pr]
                        e = epool.tile([P, 1024], bf16, tag="e")
                        slot = slot_base + pr
                        if s in SPECIAL_SPANS:
                            # holds this row-block's excluded columns:
                            # masked denominator on DVE
                            nc.scalar.activation(e[:], ps[:], EXP,
                                                 scale=MM_SCALE)
                            scr = spool.tile([P, 1024], bf16, tag="gscr")
                            o_m = 1024 - 128 * m
                            nc.vector.scalar_tensor_tensor(
                                out=scr[:],
                                in0=e[:],
                                scalar=1.0,
                                in1=masks_sb[:, o_m:o_m + 1024],
                                op0=MUL,
                                op1=MUL,
                                accum_out=den_parts[:, m, slot:slot + 1],
                            )
                        else:
                            nc.scalar.activation(
                                e[:], ps[:], EXP, scale=MM_SCALE,
                                accum_out=den_parts[:, m, slot:slot + 1],
                            )
                        # numerator: sum(e * w) over this pair's columns
                        wc0 = half_off + pr * 1024
                        scr2 = spool.tile([P, 1024], bf16, tag="scr")
                        nc.vector.scalar_tensor_tensor(
                            out=scr2[:],
                            in0=e[:],
                            scalar=1.0,
                            in1=w_sb[m][:, wc0:wc0 + 1024],
                            op0=MUL,
                            op1=MUL,
                            accum_out=num_parts[:, m, slot:slot + 1],
                        )
                slot_base += npairs

            # final 8-way slot reduction happens on the host
            nc.sync.dma_start(num_d[:], num_parts[:])
            nc.sync.dma_start(den_d[:], den_parts[:])

    _split_sync_waits(nc)
    return nc


def get_nc():
    if "nc" not in _NC_CACHE:
        _NC_CACHE["nc"] = _build_nc()
    return _NC_CACHE["nc"]


def _make_masks():
    """Sliding strip: masks[p, y] = 0 where y == 1024 + p else 1."""
    mk = np.ones((P, 2048), dtype=np.float32)
    pp = np.arange(P)
    mk[pp, 1024 + pp] = 0.0
    return mk.astype(BF16)


def prep_inputs(feature1, feature2, S_weight, pre_label):
    """Build the 8 per-core input maps + host-side special terms."""
    f1 = np.ascontiguousarray(np.asarray(feature1, dtype=np.float32))
    f2 = np.ascontiguousarray(np.asarray(feature2, dtype=np.float32))
    S = np.asarray(S_weight, dtype=np.float32)
    labels = np.asarray(pre_label).astype(np.int64)

    H = np.concatenate([f1, f2], axis=0)            # [N, D] f32
    HT = np.ascontiguousarray(H.T)                  # [D, N] f32
    if USE_FP8:
        HT_bf = (HT * np.float32(FP8_SCALE)).astype(IN_NP_DT)
    else:
        HT_bf = HT.astype(IN_NP_DT)                 # [D, N]
    masks = _make_masks()

    in_maps = []
    for k in range(NCORES):
        R = RPC * k
        rho = R % B
        perm = (np.arange(B) + rho) % B             # per-half column rotation
        cols = np.concatenate([perm, B + perm])     # [N]
        rhs = np.ascontiguousarray(
            HT_bf[:, cols].reshape(KC, P, N))       # [8, 128, 8192]
        lhsT = np.ascontiguousarray(
            HT_bf[:, R:R + RPC].reshape(KC, P, RPC))  # [8, 128, 1024]
        rows = np.arange(rho, rho + RPC)
        Sp = S[rows][:, perm] + np.float32(EPS_W)   # [1024, 4096]
        cat = labels[rows][:, None] == labels[perm][None, :]
        w = np.where(cat, Sp, np.float32(0.0))
        ii = np.arange(RPC)
        w[ii, ii] = 0.0                             # special cols excluded
        w_dt = IN_NP_DT if USE_FP8 else BF16
        in_maps.append({
            "rhs": rhs,
            "lhsT": lhsT,
            "w": np.ascontiguousarray(w.astype(w_dt)),
            "masks": masks,
        })

    # host-side special terms in float64
    H64 = H.astype(np.float64)
    sim_ii = np.einsum("ij,ij->i", H64, H64)            # [N] ~ 1.0
    cross = np.einsum("ij,ij->i", f1.astype(np.float64),
                      f2.astype(np.float64))            # [B]
    e_ii = np.exp(sim_ii * T_INV)
    e_cross = np.exp(np.concatenate([cross, cross]) * T_INV)
    s_ii = S.diagonal().astype(np.float64)
    s_cross = np.concatenate([s_ii, s_ii])
    num_special = EPS_W * e_ii + (s_cross + EPS_W) * e_cross  # [N]
    return in_maps, num_special


def postprocess(results, num_special):
    num = np.concatenate(
        [np.asarray(r["num_out"], dtype=np.float64).sum(-1).T.reshape(-1)
         for r in results])
    den = np.concatenate(
        [np.asarray(r["den_out"], dtype=np.float64).sum(-1).T.reshape(-1)
         for r in results])
    num_total = num.astype(np.float64) + num_special
    loss = -np.mean(np.log(num_total / den.astype(np.float64)))
    return np.float32(loss)


def kernel(feature1, feature2, S_weight, pre_label):
    from concourse.bass_utils import run_bass_kernel_spmd

    nc = get_nc()
    in_maps, num_special = prep_inputs(feature1, feature2, S_weight, pre_label)
    res = run_bass_kernel_spmd(nc, in_maps, core_ids=list(range(NCORES)))
    return postprocess(res.results, num_special)
